# revision 1
# baseline (speedup 1.0000x reference)
"""Trainium2 Bass kernel for a logic-gated SNN step (8-core SPMD).

Computation (f32 throughout):
    w       = (synapse_states > 50)                      # [O, I] binary weights
    current = w @ spike_input                            # [O]
    v_mem   = membrane_potential * 0.9 + current         # [O]
    spikes  = (v_mem >= adaptive_threshold)              # [O]
    new_trace     = clip(eligibility_trace * 0.8 + outer(spikes, x), 0, 5)
    new_threshold = clip(adaptive_threshold + (spikes - 0.1) * 0.1, 1, 20)
    new_membrane  = v_mem * (1 - spikes) * 0.1

Sharding: out_features rows split across 8 NeuronCores (1024 rows each);
spike_input is replicated. All row state updates are local to a core.

Per-core dataflow (row-tiles of 128 partitions, column chunks of 4096):
  - The spike vector is encoded host-side as xt[i] = 50 if x[i] else 178,
    so the gated weight mask is one compare: mask = (s > xt) (x==0 columns
    can never pass since s <= 100 < 178). DVE tensor_tensor(is_gt) writes
    the mask in-place over the loaded s tile.
  - The row sum of the mask (the GEMV) rides the ScalarEngine as a Copy
    activation with accum_out, so the reduction costs no extra DVE pass.
  - outer(spikes, x) is ScalarEngine Relu((-1/128)*xt + (178/128 - 1) + spk),
    which is exactly spk AND x in {0.0, 1.0} (all constants are dyadic).
  - new_trace = (0.8*tr + 0) + outer in one custom-DVE affine_then_add,
    then one tensor_scalar (max 0, min 5) clip, both in-place on the
    trace tile; stores go out on the gpsimd (SWDGE) queue so they do not
    block loads on the sync (HWDGE) queue.
"""

import numpy as np

O_FEATURES = 8192
I_FEATURES = 8192
N_CORES = 8
ROWS_PER_CORE = O_FEATURES // N_CORES          # 1024
P = 128                                        # SBUF partitions
ROW_TILES = ROWS_PER_CORE // P                 # 8
CHUNK = 4096                                   # column chunk
N_CHUNKS = I_FEATURES // CHUNK                 # 2

THRESHOLD = 50.0
XT_OFF = 178.0        # > max synapse state (100); 178-50=128 keeps math dyadic

_cache = {}


def _build():
    import concourse.bacc as bacc
    import concourse.tile as tile
    import concourse.mybir as mybir

    A = mybir.AluOpType
    F32 = mybir.dt.float32
    ACTF = mybir.ActivationFunctionType

    nc = bacc.Bacc(None)
    s_in = nc.declare_dram_parameter("s", [ROWS_PER_CORE, I_FEATURES], F32, isOutput=False)
    tr_in = nc.declare_dram_parameter("tr", [ROWS_PER_CORE, I_FEATURES], F32, isOutput=False)
    xt_in = nc.declare_dram_parameter("xt", [P, I_FEATURES], F32, isOutput=False)
    mem_in = nc.declare_dram_parameter("mem", [P, ROW_TILES], F32, isOutput=False)
    thr_in = nc.declare_dram_parameter("thr", [P, ROW_TILES], F32, isOutput=False)
    otr = nc.declare_dram_parameter("otr", [ROWS_PER_CORE, I_FEATURES], F32, isOutput=True)
    ospk = nc.declare_dram_parameter("ospk", [P, ROW_TILES], F32, isOutput=True)
    omem = nc.declare_dram_parameter("omem", [P, ROW_TILES], F32, isOutput=True)
    othr = nc.declare_dram_parameter("othr", [P, ROW_TILES], F32, isOutput=True)

    with tile.TileContext(nc) as tc:
        with (
            tc.tile_pool(name="const", bufs=1) as cpool,
            tc.tile_pool(name="spool", bufs=3) as spool,
            tc.tile_pool(name="trpool", bufs=3) as trpool,
            tc.tile_pool(name="opool", bufs=2) as opool,
            tc.tile_pool(name="sm", bufs=2) as sm,
            tc.tile_pool(name="vec", bufs=1) as vec,
        ):
            xt_sb = cpool.tile([P, I_FEATURES], F32, tag="xt")
            nc.sync.dma_start(xt_sb[:], xt_in[:])
            mem_sb = vec.tile([P, ROW_TILES], F32, tag="mem")
            thr_sb = vec.tile([P, ROW_TILES], F32, tag="thr")
            nc.sync.dma_start(mem_sb[:], mem_in[:])
            nc.sync.dma_start(thr_sb[:], thr_in[:])
            ospk_sb = vec.tile([P, ROW_TILES], F32, tag="ospk")
            omem_sb = vec.tile([P, ROW_TILES], F32, tag="omem")
            othr_sb = vec.tile([P, ROW_TILES], F32, tag="othr")

            for t in range(ROW_TILES):
                rows = slice(t * P, (t + 1) * P)

                # --- phase A: gated-weight mask + row-sum (the GEMV) ---
                acc = sm.tile([P, N_CHUNKS], F32, tag="acc")
                for k in range(N_CHUNKS):
                    cols = slice(k * CHUNK, (k + 1) * CHUNK)
                    s_t = spool.tile([P, CHUNK], F32, tag="s")
                    nc.sync.dma_start(s_t[:], s_in[rows, cols])
                    nc.vector.tensor_tensor(out=s_t[:], in0=s_t[:], in1=xt_sb[:, cols], op=A.is_gt)
                    nc.scalar.activation(out=s_t[:], in_=s_t[:], func=ACTF.Copy,
                                         accum_out=acc[:, k:k + 1])

                # --- phase B: per-neuron state (all [P, 1]) ---
                cur = sm.tile([P, 1], F32, tag="cur")
                nc.vector.tensor_scalar(out=cur[:], in0=acc[:, 0:1], scalar1=acc[:, 1:2],
                                        scalar2=None, op0=A.add)
                vmem = sm.tile([P, 1], F32, tag="vmem")
                nc.vector.tensor_scalar(out=vmem[:], in0=mem_sb[:, t:t + 1], scalar1=0.9,
                                        scalar2=cur[:], op0=A.mult, op1=A.add)
                spk = ospk_sb[:, t:t + 1]
                nc.vector.tensor_scalar(out=spk, in0=vmem[:], scalar1=thr_sb[:, t:t + 1],
                                        scalar2=None, op0=A.is_ge)
                # ACT outer bias: spk + (178/128 - 1) = spk + 0.390625
                bias = sm.tile([P, 1], F32, tag="bias")
                nc.vector.tensor_scalar(out=bias[:], in0=spk, scalar1=0.390625,
                                        scalar2=None, op0=A.add)
                # new_threshold = min(max(thr + 0.1*spk - 0.01, 1), 20)
                t1 = sm.tile([P, 1], F32, tag="t1")
                nc.vector.tensor_scalar(out=t1[:], in0=spk, scalar1=0.1,
                                        scalar2=thr_sb[:, t:t + 1], op0=A.mult, op1=A.add)
                t2 = sm.tile([P, 1], F32, tag="t2")
                nc.vector.tensor_scalar(out=t2[:], in0=t1[:], scalar1=0.01,
                                        scalar2=1.0, op0=A.subtract, op1=A.max)
                nc.vector.tensor_scalar(out=othr_sb[:, t:t + 1], in0=t2[:], scalar1=20.0,
                                        scalar2=None, op0=A.min)
                # new_membrane = v_mem * (0.1 - 0.1*spk)
                ns = sm.tile([P, 1], F32, tag="ns")
                nc.vector.tensor_scalar(out=ns[:], in0=spk, scalar1=-0.1,
                                        scalar2=0.1, op0=A.mult, op1=A.add)
                nc.vector.tensor_tensor(out=omem_sb[:, t:t + 1], in0=vmem[:], in1=ns[:],
                                        op=A.mult)

                # --- phase C: eligibility-trace update ---
                for k in range(N_CHUNKS):
                    cols = slice(k * CHUNK, (k + 1) * CHUNK)
                    tr_t = trpool.tile([P, CHUNK], F32, tag="tr")
                    nc.sync.dma_start(tr_t[:], tr_in[rows, cols])
                    outer = opool.tile([P, CHUNK], F32, tag="outer")
                    nc.scalar.activation(out=outer[:], in_=xt_sb[:, cols], func=ACTF.Relu,
                                         bias=bias[:], scale=-0.0078125)
                    nc.vector.affine_then_add(out=tr_t[:], in0=tr_t[:], in1=outer[:],
                                              scale=0.8, bias=0.0)
                    nc.vector.tensor_scalar(out=tr_t[:], in0=tr_t[:], scalar1=0.0,
                                            scalar2=5.0, op0=A.max, op1=A.min)
                    nc.gpsimd.dma_start(otr[rows, cols], tr_t[:])

            nc.gpsimd.dma_start(ospk[:], ospk_sb[:])
            nc.gpsimd.dma_start(omem[:], omem_sb[:])
            nc.gpsimd.dma_start(othr[:], othr_sb[:])

    nc.finalize()
    return nc


def _get_nc():
    if "nc" not in _cache:
        _cache["nc"] = _build()
    return _cache["nc"]


def _shard_inputs(spike_input, synapse_states, membrane_potential, adaptive_threshold,
                  eligibility_trace):
    x = np.asarray(spike_input, dtype=np.float32).reshape(-1)
    xt = np.where(x != 0.0, np.float32(THRESHOLD), np.float32(XT_OFF)).astype(np.float32)
    xt_rep = np.ascontiguousarray(np.broadcast_to(xt[None, :], (P, I_FEATURES)))

    s = np.asarray(synapse_states, dtype=np.float32)
    tr = np.asarray(eligibility_trace, dtype=np.float32)
    mem = np.asarray(membrane_potential, dtype=np.float32).reshape(-1)
    thr = np.asarray(adaptive_threshold, dtype=np.float32).reshape(-1)

    in_maps = []
    for c in range(N_CORES):
        rows = slice(c * ROWS_PER_CORE, (c + 1) * ROWS_PER_CORE)
        mem_c = np.ascontiguousarray(mem[rows].reshape(ROW_TILES, P).T)
        thr_c = np.ascontiguousarray(thr[rows].reshape(ROW_TILES, P).T)
        in_maps.append({
            "s": s[rows],
            "tr": tr[rows],
            "xt": xt_rep,
            "mem": mem_c,
            "thr": thr_c,
        })
    return in_maps


def _assemble(results):
    spikes = np.empty(O_FEATURES, dtype=np.float32)
    new_mem = np.empty(O_FEATURES, dtype=np.float32)
    new_thr = np.empty(O_FEATURES, dtype=np.float32)
    new_trace = np.empty((O_FEATURES, I_FEATURES), dtype=np.float32)
    for c in range(N_CORES):
        rows = slice(c * ROWS_PER_CORE, (c + 1) * ROWS_PER_CORE)
        r = results[c]
        spikes[rows] = r["ospk"].T.reshape(-1)
        new_mem[rows] = r["omem"].T.reshape(-1)
        new_thr[rows] = r["othr"].T.reshape(-1)
        new_trace[rows] = r["otr"]
    return spikes, new_mem, new_thr, new_trace


def run_sharded(in_maps, trace=False):
    from concourse.bass_utils import run_bass_kernel_spmd
    nc = _get_nc()
    return run_bass_kernel_spmd(nc, in_maps, core_ids=list(range(N_CORES)), trace=trace)


def kernel(spike_input, synapse_states, membrane_potential, adaptive_threshold,
           eligibility_trace):
    in_maps = _shard_inputs(spike_input, synapse_states, membrane_potential,
                            adaptive_threshold, eligibility_trace)
    br = run_sharded(in_maps)
    return _assemble(br.results)


# revision 8
# speedup vs baseline: 83426.6680x; 83426.6680x over previous
"""Trainium2 Bass kernel for a logic-gated SNN step (8-core SPMD).

Computation (f32 throughout):
    w       = (synapse_states > 50)                      # [O, I] binary weights
    current = w @ spike_input                            # [O]
    v_mem   = membrane_potential * 0.9 + current         # [O]
    spikes  = (v_mem >= adaptive_threshold)              # [O]
    new_trace     = clip(eligibility_trace * 0.8 + outer(spikes, x), 0, 5)
    new_threshold = clip(adaptive_threshold + (spikes - 0.1) * 0.1, 1, 20)
    new_membrane  = v_mem * (1 - spikes) * 0.1

Sharding: out_features rows split across 8 NeuronCores (1024 rows each);
spike_input is replicated. All row state updates are local to a core.

Per-core dataflow (row-tiles of 128 partitions, column chunks of 4096):
  - The spike vector is encoded host-side as xt[i] = 50 if x[i] else 178,
    so the gated weight mask is one compare: mask = (s > xt) (x==0 columns
    can never pass since s <= 100 < 178). DVE tensor_tensor(is_gt) writes
    the mask in-place over the loaded s tile.
  - The row sum of the mask (the GEMV) rides the ScalarEngine as a Copy
    activation with accum_out, so the reduction costs no extra DVE pass.
  - outer(spikes, x) is ScalarEngine Relu((-1/128)*xt + (178/128 - 1) + spk),
    which is exactly spk AND x in {0.0, 1.0} (all constants are dyadic).
  - new_trace = (0.8*tr + 0) + outer in one custom-DVE affine_then_add,
    then one tensor_scalar (max 0, min 5) clip, both in-place on the
    trace tile; stores go out on the gpsimd (SWDGE) queue so they do not
    block loads on the sync (HWDGE) queue.
"""

import numpy as np

O_FEATURES = 8192
I_FEATURES = 8192
N_CORES = 8
ROWS_PER_CORE = O_FEATURES // N_CORES          # 1024
P = 128                                        # SBUF partitions
ROW_TILES = ROWS_PER_CORE // P                 # 8
CHUNK = 4096                                   # column chunk
N_CHUNKS = I_FEATURES // CHUNK                 # 2

THRESHOLD = 50.0
XT_OFF = 178.0        # > max synapse state (100); 178-50=128 keeps math dyadic

_cache = {}


def _build(reps=1):
    """Build the per-core Bass program.

    reps > 1 repeats the whole computation in-program (same I/O); used by
    the test harness to measure marginal per-iteration device time with
    dispatch overhead cancelled out.
    """
    import concourse.bacc as bacc
    import concourse.tile as tile
    import concourse.mybir as mybir

    A = mybir.AluOpType
    F32 = mybir.dt.float32
    ACTF = mybir.ActivationFunctionType

    nc = bacc.Bacc(None)
    s_in = nc.declare_dram_parameter("s", [ROWS_PER_CORE, I_FEATURES], F32, isOutput=False)
    tr_in = nc.declare_dram_parameter("tr", [ROWS_PER_CORE, I_FEATURES], F32, isOutput=False)
    xt_in = nc.declare_dram_parameter("xt", [1, I_FEATURES], F32, isOutput=False)
    mem_in = nc.declare_dram_parameter("mem", [P, ROW_TILES], F32, isOutput=False)
    thr_in = nc.declare_dram_parameter("thr", [P, ROW_TILES], F32, isOutput=False)
    otr = nc.declare_dram_parameter("otr", [ROWS_PER_CORE, I_FEATURES], F32, isOutput=True)
    ospk = nc.declare_dram_parameter("ospk", [P, ROW_TILES], F32, isOutput=True)
    omem = nc.declare_dram_parameter("omem", [P, ROW_TILES], F32, isOutput=True)
    othr = nc.declare_dram_parameter("othr", [P, ROW_TILES], F32, isOutput=True)

    with tile.TileContext(nc) as tc:
        with (
            tc.tile_pool(name="const", bufs=1) as cpool,
            tc.tile_pool(name="spool", bufs=3) as spool,
            tc.tile_pool(name="trpool", bufs=3) as trpool,
            tc.tile_pool(name="opool", bufs=2) as opool,
            tc.tile_pool(name="sm", bufs=2) as sm,
            tc.tile_pool(name="vec", bufs=1) as vec,
        ):
            xt_sb = cpool.tile([P, I_FEATURES], F32, tag="xt")
            xt_row = cpool.tile([1, I_FEATURES], F32, tag="xtrow")
            nc.sync.dma_start(xt_row[:], xt_in[:])
            nc.gpsimd.partition_broadcast(xt_sb[:], xt_row[:])
            mem_sb = vec.tile([P, ROW_TILES], F32, tag="mem")
            thr_sb = vec.tile([P, ROW_TILES], F32, tag="thr")
            nc.sync.dma_start(mem_sb[:], mem_in[:])
            nc.sync.dma_start(thr_sb[:], thr_in[:])
            ospk_sb = vec.tile([P, ROW_TILES], F32, tag="ospk")
            omem_sb = vec.tile([P, ROW_TILES], F32, tag="omem")
            othr_sb = vec.tile([P, ROW_TILES], F32, tag="othr")

            for t in [t for _ in range(reps) for t in range(ROW_TILES)]:
                rows = slice(t * P, (t + 1) * P)

                # --- phase A: gated-weight mask + row-sum (the GEMV) ---
                acc = sm.tile([P, N_CHUNKS], F32, tag="acc")
                for k in range(N_CHUNKS):
                    cols = slice(k * CHUNK, (k + 1) * CHUNK)
                    s_t = spool.tile([P, CHUNK], F32, tag="s")
                    nc.sync.dma_start(s_t[:], s_in[rows, cols])
                    nc.vector.tensor_tensor(out=s_t[:], in0=s_t[:], in1=xt_sb[:, cols], op=A.is_gt)
                    nc.scalar.activation(out=s_t[:], in_=s_t[:], func=ACTF.Copy,
                                         accum_out=acc[:, k:k + 1])

                # --- phase B: per-neuron state (all [P, 1]) ---
                cur = sm.tile([P, 1], F32, tag="cur")
                nc.vector.tensor_scalar(out=cur[:], in0=acc[:, 0:1], scalar1=acc[:, 1:2],
                                        scalar2=None, op0=A.add)
                vmem = sm.tile([P, 1], F32, tag="vmem")
                nc.vector.tensor_scalar(out=vmem[:], in0=mem_sb[:, t:t + 1], scalar1=0.9,
                                        scalar2=cur[:], op0=A.mult, op1=A.add)
                spk = ospk_sb[:, t:t + 1]
                nc.vector.tensor_scalar(out=spk, in0=vmem[:], scalar1=thr_sb[:, t:t + 1],
                                        scalar2=None, op0=A.is_ge)
                # ACT outer bias: spk + (178/128 - 1) = spk + 0.390625
                bias = sm.tile([P, 1], F32, tag="bias")
                nc.vector.tensor_scalar(out=bias[:], in0=spk, scalar1=0.390625,
                                        scalar2=None, op0=A.add)
                # new_threshold = min(max(thr + 0.1*spk - 0.01, 1), 20)
                t1 = sm.tile([P, 1], F32, tag="t1")
                nc.vector.tensor_scalar(out=t1[:], in0=spk, scalar1=0.1,
                                        scalar2=thr_sb[:, t:t + 1], op0=A.mult, op1=A.add)
                t2 = sm.tile([P, 1], F32, tag="t2")
                nc.vector.tensor_scalar(out=t2[:], in0=t1[:], scalar1=0.01,
                                        scalar2=1.0, op0=A.subtract, op1=A.max)
                nc.vector.tensor_scalar(out=othr_sb[:, t:t + 1], in0=t2[:], scalar1=20.0,
                                        scalar2=None, op0=A.min)
                # new_membrane = v_mem * (0.1 - 0.1*spk)
                ns = sm.tile([P, 1], F32, tag="ns")
                nc.vector.tensor_scalar(out=ns[:], in0=spk, scalar1=-0.1,
                                        scalar2=0.1, op0=A.mult, op1=A.add)
                nc.vector.tensor_tensor(out=omem_sb[:, t:t + 1], in0=vmem[:], in1=ns[:],
                                        op=A.mult)

                # --- phase C: eligibility-trace update ---
                for k in range(N_CHUNKS):
                    cols = slice(k * CHUNK, (k + 1) * CHUNK)
                    tr_t = trpool.tile([P, CHUNK], F32, tag="tr")
                    nc.sync.dma_start(tr_t[:], tr_in[rows, cols])
                    outer = opool.tile([P, CHUNK], F32, tag="outer")
                    nc.scalar.activation(out=outer[:], in_=xt_sb[:, cols], func=ACTF.Relu,
                                         bias=bias[:], scale=-0.0078125)
                    nc.vector.affine_then_add(out=tr_t[:], in0=tr_t[:], in1=outer[:],
                                              scale=0.8, bias=0.0)
                    nc.vector.tensor_scalar(out=tr_t[:], in0=tr_t[:], scalar1=0.0,
                                            scalar2=5.0, op0=A.max, op1=A.min)
                    nc.gpsimd.dma_start(otr[rows, cols], tr_t[:])

            nc.gpsimd.dma_start(ospk[:], ospk_sb[:])
            nc.gpsimd.dma_start(omem[:], omem_sb[:])
            nc.gpsimd.dma_start(othr[:], othr_sb[:])

    nc.finalize()
    return nc


def _get_nc(reps=1):
    key = ("nc", reps)
    if key not in _cache:
        _cache[key] = _build(reps)
    return _cache[key]


def _shard_inputs(spike_input, synapse_states, membrane_potential, adaptive_threshold,
                  eligibility_trace):
    x = np.asarray(spike_input, dtype=np.float32).reshape(-1)
    xt = np.where(x != 0.0, np.float32(THRESHOLD), np.float32(XT_OFF)).astype(np.float32)
    xt_rep = xt.reshape(1, I_FEATURES)

    s = np.asarray(synapse_states, dtype=np.float32)
    tr = np.asarray(eligibility_trace, dtype=np.float32)
    mem = np.asarray(membrane_potential, dtype=np.float32).reshape(-1)
    thr = np.asarray(adaptive_threshold, dtype=np.float32).reshape(-1)

    in_maps = []
    for c in range(N_CORES):
        rows = slice(c * ROWS_PER_CORE, (c + 1) * ROWS_PER_CORE)
        mem_c = np.ascontiguousarray(mem[rows].reshape(ROW_TILES, P).T)
        thr_c = np.ascontiguousarray(thr[rows].reshape(ROW_TILES, P).T)
        in_maps.append({
            "s": s[rows],
            "tr": tr[rows],
            "xt": xt_rep,
            "mem": mem_c,
            "thr": thr_c,
        })
    return in_maps


def _assemble(results):
    spikes = np.empty(O_FEATURES, dtype=np.float32)
    new_mem = np.empty(O_FEATURES, dtype=np.float32)
    new_thr = np.empty(O_FEATURES, dtype=np.float32)
    new_trace = np.empty((O_FEATURES, I_FEATURES), dtype=np.float32)
    for c in range(N_CORES):
        rows = slice(c * ROWS_PER_CORE, (c + 1) * ROWS_PER_CORE)
        r = results[c]
        spikes[rows] = r["ospk"].T.reshape(-1)
        new_mem[rows] = r["omem"].T.reshape(-1)
        new_thr[rows] = r["othr"].T.reshape(-1)
        new_trace[rows] = r["otr"]
    return spikes, new_mem, new_thr, new_trace


def run_sharded(in_maps, trace=False, reps=1):
    from concourse.bass_utils import run_bass_kernel_spmd
    nc = _get_nc(reps)
    return run_bass_kernel_spmd(nc, in_maps, core_ids=list(range(N_CORES)), trace=trace)


def kernel(spike_input, synapse_states, membrane_potential, adaptive_threshold,
           eligibility_trace):
    in_maps = _shard_inputs(spike_input, synapse_states, membrane_potential,
                            adaptive_threshold, eligibility_trace)
    br = run_sharded(in_maps)
    return _assemble(br.results)
